# revision 11
# baseline (speedup 1.0000x reference)
"""Trainium2 Bass kernel for the GNN message-passing rollout net.

Strategy: pure data-parallel over batch (1024 -> 8 cores x 128), everything
feature-major on-chip ([features, tokens]); the o*o pairwise tensor is never
materialized - broadcast access patterns feed the first-layer matmuls, the
attention weight is folded into the relation output by linearity, and the
j-reduction happens *before* (not after) the last relation layer.
"""

import sys

for p in ("/opt/trn_rl_repo",):
    if p not in sys.path:
        sys.path.insert(0, p)

import numpy as np

import concourse.bass as bass
import concourse.tile as tile
from concourse import bacc, mybir
from concourse.bass_utils import run_bass_kernel_spmd

F32 = mybir.dt.float32
AF = mybir.ActivationFunctionType
ALU = mybir.AluOpType

NCORES = 8
B = 128          # batch per core
O = 16           # objects
CL = 64          # code length
NT = B * O       # 2048 (b,o) tokens per core
CP = 512         # pairwise chunk (columns per matmul)
NCHUNK = (B * O * O) // CP   # 64 pairwise chunks per step
NGRP = 4         # bi-level groups of 512 tokens
GW = NT // NGRP  # 512
CPG = NCHUNK // NGRP  # 16 pairwise chunks per group

_CACHE = {}


def _ap(t, extra, off=0):
    """AP with tile's partition dim + custom free dims (element units)."""
    return bass.AP(tensor=t.tensor, offset=t.offset + off,
                   ap=[list(t.ap[0])] + extra)


def build_program(num_rollout: int, b: int = B, ncores: int = NCORES):
    NT = b * O
    NCHUNK = (b * O * O) // CP
    GW = min(512, NT)
    NGRP = NT // GW
    CPG = NCHUNK // NGRP
    nc = bacc.Bacc("TRN2", target_bir_lowering=False, debug=False,
                   num_devices=ncores)

    d_s0 = nc.dram_tensor("s0", [CL, NT], F32, kind="ExternalInput").ap()
    d_wrel0 = nc.dram_tensor("wrel0", [CL, 256], F32, kind="ExternalInput").ap()
    d_watt0 = nc.dram_tensor("watt0", [CL, 256], F32, kind="ExternalInput").ap()
    d_wdist = nc.dram_tensor("wdist", [1, 256], F32, kind="ExternalInput").ap()
    d_w1 = nc.dram_tensor("w1", [128, 128], F32, kind="ExternalInput").ap()
    d_whi = nc.dram_tensor("whi", [CL, CL + 1], F32, kind="ExternalInput").ap()
    d_wqe = nc.dram_tensor("wqe", [CL + 1, CL], F32, kind="ExternalInput").ap()
    d_wbi = nc.dram_tensor("wbi", [CL, 8 * CL], F32, kind="ExternalInput").ap()
    d_bias = nc.dram_tensor("bias", [128, 10], F32, kind="ExternalInput").ap()
    d_ro = nc.dram_tensor("ro", [num_rollout, 4, NT], F32,
                          kind="ExternalOutput").ap()

    with tile.TileContext(nc) as tc, tc.tile_pool(name="wp", bufs=1) as wp, \
            tc.tile_pool(name="sp", bufs=2) as sp, \
            tc.tile_pool(name="dp", bufs=2) as dpool, \
            tc.tile_pool(name="drow", bufs=2) as drow_p, \
            tc.tile_pool(name="qp", bufs=2) as qp, \
            tc.tile_pool(name="pw", bufs=3) as pw, \
            tc.tile_pool(name="bi", bufs=2) as bi, \
            tc.tile_pool(name="ps", bufs=8, space="PSUM") as ps:

        def pst(parts=128):
            return ps.tile([parts, CP], F32, tag="ps", name="pst")

        # ---- load weights (once) ----
        w_rel0 = wp.tile([CL, 256], F32); nc.sync.dma_start(w_rel0, d_wrel0)
        w_att0 = wp.tile([CL, 256], F32); nc.sync.dma_start(w_att0, d_watt0)
        w_dist = wp.tile([1, 256], F32); nc.sync.dma_start(w_dist, d_wdist)
        w_1 = wp.tile([128, 128], F32); nc.sync.dma_start(w_1, d_w1)
        w_hi = wp.tile([CL, CL + 1], F32); nc.sync.dma_start(w_hi, d_whi)
        w_qe = wp.tile([CL + 1, CL], F32); nc.sync.dma_start(w_qe, d_wqe)
        w_bi = wp.tile([CL, 8 * CL], F32); nc.sync.dma_start(w_bi, d_wbi)
        bias = wp.tile([128, 10], F32); nc.sync.dma_start(bias, d_bias)

        sc0W_s = w_bi[:, 0:CL]
        wsc_aff = w_bi[:, CL:2 * CL]
        aff1W_s = w_bi[:, 3 * CL:4 * CL]
        waff2_out = w_bi[:, 4 * CL:5 * CL]
        out0bW_s = w_bi[:, 5 * CL:6 * CL]
        w_out1 = w_bi[:, 6 * CL:7 * CL]
        p01 = w_bi[:, 7 * CL:8 * CL]

        b_rel0 = bias[:, 0:1]
        b_att0 = bias[:, 1:2]
        b_p2e = bias[0:CL + 1, 2:3]
        b_sc0 = bias[0:CL, 3:4]
        b_aff = bias[0:CL, 4:5]
        b_aff1 = bias[0:CL, 5:6]
        b_u1 = bias[0:CL, 6:7]
        b_out1 = bias[0:CL, 7:8]
        b_eb = bias[0:CL + 1, 8:9]
        b_att1 = bias[0:CL, 9:10]
        ones_row = wp.tile([1, CP], F32)
        nc.vector.memset(ones_row, 1.0)

        S = sp.tile([CL, NT], F32, tag="S")
        nc.sync.dma_start(S, d_s0)

        for t in range(num_rollout):
            # ---- token-major coordinate rows + pairwise distances ----
            XI = dpool.tile([NCHUNK, 32], F32, tag="XI")
            YI = dpool.tile([NCHUNK, 32], F32, tag="YI")
            nc.sync.dma_start(XI, _ap(S, [[32, NCHUNK], [1, 32]])[0:1])
            nc.sync.dma_start(YI, _ap(S, [[32, NCHUNK], [1, 32]], off=NT)[0:1])

            def bj(tl):  # (b,i,j) -> col 16b+i
                return _ap(tl, [[16, 2], [1, 16], [0, 16]])

            def bi_(tl):  # (b,i,j) -> col 16b+j
                return _ap(tl, [[16, 2], [0, 16], [1, 16]])

            dx = dpool.tile([NCHUNK, CP], F32, tag="dx")
            dy = dpool.tile([NCHUNK, CP], F32, tag="dy")
            nc.vector.tensor_sub(dx, bj(XI), bi_(XI))
            nc.vector.tensor_sub(dy, bj(YI), bi_(YI))
            sqx = dpool.tile([NCHUNK, CP], F32, tag="sqx")
            sqy = dpool.tile([NCHUNK, CP], F32, tag="sqy")
            nc.scalar.square(sqx, dx)
            nc.scalar.square(sqy, dy)
            distc = dpool.tile([NCHUNK, CP], F32, tag="distc")
            nc.vector.tensor_add(distc, sqx, sqy)

            Snext = sp.tile([CL, NT], F32, tag="S")
            Q = qp.tile([CL + 1, NT], F32, tag="Q")

            for g in range(NGRP):
                dist_row = drow_p.tile([1, CPG * CP], F32, tag="dist_row")
                nc.sync.dma_start(dist_row, distc[CPG * g:CPG * (g + 1), :])

                for cc in range(CPG):
                    c = CPG * g + cc
                    soff = 32 * c
                    s_bj = _ap(S, [[16, 2], [1, 16], [0, 16]], off=soff)
                    s_bi = _ap(S, [[16, 2], [0, 16], [1, 16]], off=soff)
                    dr = dist_row[0:1, CP * cc:CP * (cc + 1)]

                    pAr = pst()
                    nc.tensor.matmul(pAr, w_rel0[:, 0:128], s_bj,
                                     start=True, stop=False)
                    nc.tensor.matmul(pAr, w_rel0[:, 128:256], s_bi,
                                     start=False, stop=False)
                    nc.tensor.matmul(pAr, w_dist[0:1, 0:128], dr,
                                     start=False, stop=True)
                    pAa = pst()
                    nc.tensor.matmul(pAa, w_att0[:, 0:128], s_bj,
                                     start=True, stop=False)
                    nc.tensor.matmul(pAa, w_att0[:, 128:256], s_bi,
                                     start=False, stop=False)
                    nc.tensor.matmul(pAa, w_dist[0:1, 128:256], dr,
                                     start=False, stop=True)

                    r1 = pw.tile([128, CP], F32, tag="r1")
                    nc.scalar.activation(r1, pAr, AF.Relu, bias=b_rel0)
                    a1 = pw.tile([128, CP], F32, tag="a1")
                    nc.vector.tensor_scalar(a1, pAa, b_att0, 0.0,
                                            op0=ALU.add, op1=ALU.max)

                    p2 = pst(CL + 1)
                    nc.tensor.matmul(p2[0:CL], w_1[:, 0:CL], r1,
                                     start=True, stop=True,
                                     tile_position=(0, 0))
                    nc.tensor.matmul(p2[CL:CL + 1], ones_row[0:1, 0:1],
                                     ones_row, start=True, stop=True,
                                     tile_position=(0, 64))
                    pa2 = pst(CL)
                    nc.tensor.matmul(pa2, w_1[:, CL:128], a1,
                                     start=True, stop=True)
                    r2e = pw.tile([CL + 1, CP], F32, tag="r2e")
                    nc.scalar.activation(r2e, p2, AF.Relu, bias=b_p2e)
                    a2 = pw.tile([CL, CP], F32, tag="a2")
                    nc.vector.tensor_scalar(a2, pa2, b_att1, 0.0,
                                            op0=ALU.add, op1=ALU.max)

                    p4 = pst(CL + 1)
                    nc.tensor.matmul(p4, w_hi, a2, start=True, stop=True)
                    asb = pw.tile([CL + 1, CP], F32, tag="asb")
                    nc.scalar.activation(asb, p4, AF.Exp, bias=b_eb)
                    T = pw.tile([CL + 1, CP], F32, tag="T")
                    nc.gpsimd.tensor_mul(T, r2e, asb)

                    tmp = pw.tile([CL + 1, 32], F32, tag="tmp")
                    nc.vector.tensor_reduce(
                        tmp, T.rearrange("p (g j) -> p g j", j=16),
                        axis=mybir.AxisListType.X, op=ALU.add)
                    t_diag = _ap(T, [[256, 2], [17, 16]])
                    nc.vector.tensor_sub(Q[:, soff:soff + 32], tmp, t_diag)

                # ---- bi-level (self + affector + out) for group g ----
                cols = slice(GW * g, GW * (g + 1))
                ph = pst(CL)[:, 0:GW]
                nc.tensor.matmul(ph, sc0W_s, S[:, cols], start=True, stop=True)
                h1 = bi.tile([CL, GW], F32, tag="h1")
                nc.scalar.activation(h1, ph, AF.Relu, bias=b_sc0)

                pf1 = pst(CL)[:, 0:GW]
                nc.tensor.matmul(pf1, wsc_aff, h1, start=True, stop=False)
                nc.tensor.matmul(pf1, w_qe, Q[0:CL + 1, cols],
                                 start=False, stop=True)
                f1 = bi.tile([CL, GW], F32, tag="f1")
                nc.scalar.activation(f1, pf1, AF.Tanh, bias=b_aff)

                pf2 = pst(CL)[:, 0:GW]
                nc.tensor.matmul(pf2, aff1W_s, f1, start=True, stop=True)
                f2t = bi.tile([CL, GW], F32, tag="f2t")
                nc.scalar.activation(f2t, pf2, AF.Tanh, bias=b_aff1)

                pu = pst(CL)[:, 0:GW]
                nc.tensor.matmul(pu, waff2_out, f2t, start=True, stop=False)
                nc.tensor.matmul(pu, waff2_out, f1, start=False, stop=False)
                nc.tensor.matmul(pu, out0bW_s, S[:, cols],
                                 start=False, stop=True)
                u1 = bi.tile([CL, GW], F32, tag="u1")
                nc.scalar.activation(u1, pu, AF.Tanh, bias=b_u1)

                po = pst(CL)[:, 0:GW]
                nc.tensor.matmul(po, p01, S[:, cols], start=True, stop=False)
                nc.tensor.matmul(po, w_out1, u1, start=False, stop=True)
                nc.scalar.activation(Snext[:, cols], po, AF.Identity,
                                     bias=b_out1)

            nc.sync.dma_start(d_ro[t], Snext[0:4, :])
            S = Snext

    nc.compile()
    return nc


def _prep(inputs, b: int = B, ncores: int = NCORES):
    """Host-side weight preparation -> per-core input maps."""
    NT = b * O
    f = lambda k: np.asarray(inputs[k], np.float32)
    x = f("x")
    encW, encb = f("encW"), f("encb")
    I = np.eye(CL, dtype=np.float32)

    encWp = encW.copy(); encWp[:, 0:4] = np.eye(4, dtype=np.float32)
    encbp = encb.copy(); encbp[0:4] = 0.0

    rel0W, att0W = f("rel0W"), f("att0W")
    wrel0 = np.concatenate([rel0W[0:CL], rel0W[CL:128]], axis=1)      # [64,256]
    watt0 = np.concatenate([att0W[0:CL], att0W[CL:128]], axis=1)
    wdist = np.concatenate([rel0W[128:129], att0W[128:129]], axis=1)  # [1,256]
    w1 = np.concatenate([f("rel1W"), f("att1W")], axis=1)             # [128,128]

    whi = np.tile(f("att2W"), (1, CL + 1)).astype(np.float32)        # [64,65]
    aff0W = f("aff0W")
    wqe = np.zeros((CL + 1, CL), np.float32)
    wqe[0:CL] = (f("rel2W") + I) @ aff0W
    wqe[CL] = f("rel2b") @ aff0W                              # raff2

    sc1W = f("sc1W")
    aff2W, out0W, out1W = f("aff2W"), f("out0W"), f("out1W")
    wbi = np.zeros((CL, 8 * CL), np.float32)
    wbi[:, 0:CL] = f("sc0W")
    wbi[:, CL:2 * CL] = (sc1W + I) @ aff0W
    wbi[:, 3 * CL:4 * CL] = f("aff1W")
    wbi[:, 4 * CL:5 * CL] = aff2W @ out0W[0:CL]
    wbi[:, 5 * CL:6 * CL] = out0W[CL:128]
    wbi[:, 6 * CL:7 * CL] = out1W + I
    wbi[0, 7 * CL + 0] = 1.0
    wbi[1, 7 * CL + 1] = 1.0

    bias = np.zeros((128, 10), np.float32)
    bias[:, 0] = f("rel0b")
    bias[:, 1] = f("att0b")
    bias[0:CL, 2] = f("rel1b")
    bias[0:CL, 9] = f("att1b")
    bias[0:CL, 3] = f("sc0b")
    bias[0:CL, 4] = aff0W.T @ f("sc1b") + f("aff0b")
    bias[0:CL, 5] = f("aff1b")
    bias[0:CL, 6] = out0W[0:CL].T @ f("aff2b") + f("out0b")
    bias[0:CL, 7] = f("out1b")
    bias[0:CL + 1, 8] = float(f("att2b")[0])

    shared = dict(wrel0=wrel0, watt0=watt0, wdist=wdist, w1=w1, whi=whi,
                  wqe=wqe, wbi=wbi, bias=bias)

    in_maps = []
    for c in range(ncores):
        xs = x[c * b:(c + 1) * b, -1]
        xfm = xs.reshape(NT, 4).T.astype(np.float32)     # [4, 2048]
        s0 = encWp.T @ xfm + encbp[:, None]              # [64, 2048]
        in_maps.append(dict(shared, s0=np.ascontiguousarray(s0, np.float32)))
    return in_maps


def kernel(**inputs):
    num_rollout = int(inputs["num_rollout"])
    if num_rollout not in _CACHE:
        _CACHE[num_rollout] = build_program(num_rollout)
    nc = _CACHE[num_rollout]

    in_maps = _prep(inputs)
    res = run_bass_kernel_spmd(nc, in_maps, list(range(NCORES)))

    x = np.asarray(inputs["x"], np.float32)
    rollout = np.empty((NCORES * B, num_rollout, O, 4), np.float32)
    for c in range(NCORES):
        ro = res.results[c]["ro"]                        # [roll, 4, 2048]
        rollout[c * B:(c + 1) * B] = (
            ro.reshape(num_rollout, 4, B, O).transpose(2, 0, 3, 1))
    present = x.copy()
    return rollout, present


# revision 13
# speedup vs baseline: 2.8838x; 2.8838x over previous
"""Trainium2 Bass kernel for the GNN message-passing rollout net.

Strategy: pure data-parallel over batch (1024 -> 8 cores x 128), everything
feature-major on-chip ([features, tokens]); the o*o pairwise tensor is never
materialized - broadcast access patterns feed the first-layer matmuls, the
attention weight is folded into the relation output by linearity, and the
j-reduction happens *before* (not after) the last relation layer.
"""

import sys

for p in ("/opt/trn_rl_repo",):
    if p not in sys.path:
        sys.path.insert(0, p)

import numpy as np
import ml_dtypes

BF = ml_dtypes.bfloat16

import concourse.bass as bass
import concourse.tile as tile
from concourse import bacc, mybir
from concourse.bass_utils import run_bass_kernel_spmd

F32 = mybir.dt.float32
BF16 = mybir.dt.bfloat16
F32R = mybir.dt.float32r
AF = mybir.ActivationFunctionType
ALU = mybir.AluOpType

NCORES = 8
B = 128          # batch per core
O = 16           # objects
CL = 64          # code length
NT = B * O       # 2048 (b,o) tokens per core
CP = 512         # pairwise chunk (columns per matmul)
NCHUNK = (B * O * O) // CP   # 64 pairwise chunks per step
NGRP = 4         # bi-level groups of 512 tokens
GW = NT // NGRP  # 512
CPG = NCHUNK // NGRP  # 16 pairwise chunks per group

_CACHE = {}


def _ap(t, extra, off=0):
    """AP with tile's partition dim + custom free dims (element units)."""
    return bass.AP(tensor=t.tensor, offset=t.offset + off,
                   ap=[list(t.ap[0])] + extra)


def build_program(num_rollout: int, b: int = B, ncores: int = NCORES):
    NT = b * O
    NCHUNK = (b * O * O) // CP
    GW = min(512, NT)
    NGRP = NT // GW
    CPG = NCHUNK // NGRP
    nc = bacc.Bacc("TRN2", target_bir_lowering=False, debug=False,
                   num_devices=ncores)

    d_s0 = nc.dram_tensor("s0", [CL, NT], F32, kind="ExternalInput").ap()
    d_wrel0 = nc.dram_tensor("wrel0", [CL, 256], F32, kind="ExternalInput").ap()
    d_watt0 = nc.dram_tensor("watt0", [CL, 256], F32, kind="ExternalInput").ap()
    d_wdist = nc.dram_tensor("wdist", [1, 256], F32, kind="ExternalInput").ap()
    d_w1 = nc.dram_tensor("w1", [128, 129], F32, kind="ExternalInput").ap()
    d_whi = nc.dram_tensor("whi", [CL, CL + 1], F32, kind="ExternalInput").ap()
    d_wqe = nc.dram_tensor("wqe", [CL + 1, CL], F32, kind="ExternalInput").ap()
    d_wbi = nc.dram_tensor("wbi", [CL, 8 * CL], F32, kind="ExternalInput").ap()
    d_bias = nc.dram_tensor("bias", [128, 10], F32, kind="ExternalInput").ap()
    d_ro = nc.dram_tensor("ro", [num_rollout, 4, NT], F32,
                          kind="ExternalOutput").ap()

    with tile.TileContext(nc) as tc, tc.tile_pool(name="wp", bufs=1) as wp, \
            tc.tile_pool(name="sp", bufs=2) as sp, \
            tc.tile_pool(name="dp", bufs=2) as dpool, \
            tc.tile_pool(name="drow", bufs=2) as drow_p, \
            tc.tile_pool(name="qp", bufs=2) as qp, \
            tc.tile_pool(name="pw", bufs=3) as pw, \
            tc.tile_pool(name="bi", bufs=2) as bi, \
            tc.tile_pool(name="ps", bufs=8, space="PSUM") as ps:

        def pst(parts=128):
            return ps.tile([parts, CP], F32, tag="ps", name="pst")

        # ---- load weights (once) ----
        def wload(name, shape, dsrc):
            stg = wp.tile(shape, F32, name=f"{name}_stg")
            nc.sync.dma_start(stg, dsrc)
            t = wp.tile(shape, F32R, name=name)
            nc.vector.tensor_copy(t, stg)
            return t

        w_rel0 = wload("w_rel0", [CL, 256], d_wrel0)
        w_att0 = wload("w_att0", [CL, 256], d_watt0)
        w_dist = wload("w_dist", [1, 256], d_wdist)
        w_1 = wload("w_1", [128, 129], d_w1)
        w_hi = wload("w_hi", [CL, CL + 1], d_whi)
        w_qe = wload("w_qe", [CL + 1, CL], d_wqe)
        w_bi = wload("w_bi", [CL, 8 * CL], d_wbi)
        bias = wp.tile([128, 10], F32); nc.sync.dma_start(bias, d_bias)

        sc0W_s = w_bi[:, 0:CL]
        wsc_aff = w_bi[:, CL:2 * CL]
        aff1W_s = w_bi[:, 3 * CL:4 * CL]
        waff2_out = w_bi[:, 4 * CL:5 * CL]
        out0bW_s = w_bi[:, 5 * CL:6 * CL]
        w_out1 = w_bi[:, 6 * CL:7 * CL]
        p01 = w_bi[:, 7 * CL:8 * CL]

        b_rel0 = bias[:, 0:1]
        b_att0 = bias[:, 1:2]
        b_p2e = bias[0:CL + 1, 2:3]
        b_sc0 = bias[0:CL, 3:4]
        b_aff = bias[0:CL, 4:5]
        b_aff1 = bias[0:CL, 5:6]
        b_u1 = bias[0:CL, 6:7]
        b_out1 = bias[0:CL, 7:8]
        b_eb = bias[0:CL + 1, 8:9]
        b_att1 = bias[0:CL, 9:10]

        S = sp.tile([CL, NT], F32, tag="S")
        nc.sync.dma_start(S, d_s0)
        S16 = sp.tile([CL, NT], F32R, tag="S16")
        nc.vector.tensor_copy(S16, S)

        for t in range(num_rollout):
            # ---- token-major coordinate rows + pairwise distances ----
            XI = dpool.tile([NCHUNK, 32], F32, tag="XI")
            YI = dpool.tile([NCHUNK, 32], F32, tag="YI")
            nc.sync.dma_start(XI, _ap(S, [[32, NCHUNK], [1, 32]])[0:1])
            nc.sync.dma_start(YI, _ap(S, [[32, NCHUNK], [1, 32]], off=NT)[0:1])

            def bj(tl):  # (b,i,j) -> col 16b+i
                return _ap(tl, [[16, 2], [1, 16], [0, 16]])

            def bi_(tl):  # (b,i,j) -> col 16b+j
                return _ap(tl, [[16, 2], [0, 16], [1, 16]])

            dx = dpool.tile([NCHUNK, CP], F32, tag="dx")
            dy = dpool.tile([NCHUNK, CP], F32, tag="dy")
            nc.vector.tensor_sub(dx, bj(XI), bi_(XI))
            nc.vector.tensor_sub(dy, bj(YI), bi_(YI))
            sqx = dpool.tile([NCHUNK, CP], F32, tag="sqx")
            sqy = dpool.tile([NCHUNK, CP], F32, tag="sqy")
            nc.scalar.square(sqx, dx)
            nc.scalar.square(sqy, dy)
            distc = dpool.tile([NCHUNK, CP], F32R, tag="distc")
            nc.vector.tensor_add(distc, sqx, sqy)

            Snext = sp.tile([CL, NT], F32, tag="S")
            Snext16 = sp.tile([CL, NT], F32R, tag="S16")
            Q = qp.tile([CL + 1, NT], F32R, tag="Q")

            for g in range(NGRP):
                dist_row = drow_p.tile([1, CPG * CP], F32R, tag="dist_row")
                nc.sync.dma_start(dist_row, distc[CPG * g:CPG * (g + 1), :])

                for cc in range(CPG):
                    c = CPG * g + cc
                    soff = 32 * c
                    s_bj = _ap(S16, [[16, 2], [1, 16], [0, 16]], off=soff)
                    s_bi = _ap(S16, [[16, 2], [0, 16], [1, 16]], off=soff)
                    dr = dist_row[0:1, CP * cc:CP * (cc + 1)]

                    pAr = pst()
                    nc.tensor.matmul(pAr, w_rel0[:, 0:128], s_bj,
                                     start=True, stop=False)
                    nc.tensor.matmul(pAr, w_rel0[:, 128:256], s_bi,
                                     start=False, stop=False)
                    nc.tensor.matmul(pAr, w_dist[0:1, 0:128], dr,
                                     start=False, stop=True)
                    pAa = pst()
                    nc.tensor.matmul(pAa, w_att0[:, 0:128], s_bj,
                                     start=True, stop=False)
                    nc.tensor.matmul(pAa, w_att0[:, 128:256], s_bi,
                                     start=False, stop=False)
                    nc.tensor.matmul(pAa, w_dist[0:1, 128:256], dr,
                                     start=False, stop=True)

                    r1 = pw.tile([128, CP], F32R, tag="r1")
                    nc.scalar.activation(r1, pAr, AF.Relu, bias=b_rel0)
                    a1 = pw.tile([128, CP], F32R, tag="a1")
                    nc.vector.tensor_scalar(a1, pAa, b_att0, 0.0,
                                            op0=ALU.add, op1=ALU.max)

                    p2 = pst(CL + 1)
                    nc.tensor.matmul(p2, w_1[:, 0:CL + 1], r1,
                                     start=True, stop=True)
                    pa2 = pst(CL)
                    nc.tensor.matmul(pa2, w_1[:, CL + 1:129], a1,
                                     start=True, stop=True)
                    r2e = pw.tile([CL + 1, CP], F32, tag="r2e")
                    nc.scalar.activation(r2e, p2, AF.Relu, bias=b_p2e)
                    a2 = pw.tile([CL, CP], F32R, tag="a2")
                    nc.vector.tensor_scalar(a2, pa2, b_att1, 0.0,
                                            op0=ALU.add, op1=ALU.max)

                    p4 = pst(CL + 1)
                    nc.tensor.matmul(p4, w_hi, a2, start=True, stop=True)
                    asb = pw.tile([CL + 1, CP], F32, tag="asb")
                    nc.scalar.activation(asb, p4, AF.Exp, bias=b_eb)
                    T = pw.tile([CL + 1, CP], F32, tag="T")
                    nc.gpsimd.tensor_mul(T, r2e, asb)

                    tmp = pw.tile([CL + 1, 32], F32, tag="tmp")
                    nc.vector.tensor_reduce(
                        tmp, T.rearrange("p (g j) -> p g j", j=16),
                        axis=mybir.AxisListType.X, op=ALU.add)
                    t_diag = _ap(T, [[256, 2], [17, 16]])
                    nc.vector.tensor_sub(Q[:, soff:soff + 32], tmp, t_diag)

                # ---- bi-level (self + affector + out) for group g ----
                cols = slice(GW * g, GW * (g + 1))
                ph = pst(CL)[:, 0:GW]
                nc.tensor.matmul(ph, sc0W_s, S16[:, cols], start=True, stop=True)
                h1 = bi.tile([CL, GW], F32R, tag="h1")
                nc.scalar.activation(h1, ph, AF.Relu, bias=b_sc0)

                pf1 = pst(CL)[:, 0:GW]
                nc.tensor.matmul(pf1, wsc_aff, h1, start=True, stop=False)
                nc.tensor.matmul(pf1, w_qe, Q[0:CL + 1, cols],
                                 start=False, stop=True)
                f1 = bi.tile([CL, GW], F32R, tag="f1")
                nc.scalar.activation(f1, pf1, AF.Tanh, bias=b_aff)

                pf2 = pst(CL)[:, 0:GW]
                nc.tensor.matmul(pf2, aff1W_s, f1, start=True, stop=True)
                f2t = bi.tile([CL, GW], F32R, tag="f2t")
                nc.scalar.activation(f2t, pf2, AF.Tanh, bias=b_aff1)

                pu = pst(CL)[:, 0:GW]
                nc.tensor.matmul(pu, waff2_out, f2t, start=True, stop=False)
                nc.tensor.matmul(pu, waff2_out, f1, start=False, stop=False)
                nc.tensor.matmul(pu, out0bW_s, S16[:, cols],
                                 start=False, stop=True)
                u1 = bi.tile([CL, GW], F32R, tag="u1")
                nc.scalar.activation(u1, pu, AF.Tanh, bias=b_u1)

                po = pst(CL)[:, 0:GW]
                nc.tensor.matmul(po, p01, S16[:, cols], start=True, stop=False)
                nc.tensor.matmul(po, w_out1, u1, start=False, stop=True)
                nc.scalar.activation(Snext[:, cols], po, AF.Identity,
                                     bias=b_out1)
                nc.vector.tensor_copy(Snext16[:, cols], Snext[:, cols])

            nc.sync.dma_start(d_ro[t], Snext[0:4, :])
            S = Snext
            S16 = Snext16

    nc.compile()
    return nc


def _prep(inputs, b: int = B, ncores: int = NCORES):
    """Host-side weight preparation -> per-core input maps."""
    NT = b * O
    f = lambda k: np.asarray(inputs[k], np.float32)
    x = f("x")
    encW, encb = f("encW"), f("encb")
    I = np.eye(CL, dtype=np.float32)

    encWp = encW.copy(); encWp[:, 0:4] = np.eye(4, dtype=np.float32)
    encbp = encb.copy(); encbp[0:4] = 0.0

    rel0W, att0W = f("rel0W"), f("att0W")
    wrel0 = np.concatenate([rel0W[0:CL], rel0W[CL:128]], axis=1)      # [64,256]
    watt0 = np.concatenate([att0W[0:CL], att0W[CL:128]], axis=1)
    wdist = np.concatenate([rel0W[128:129], att0W[128:129]], axis=1)  # [1,256]
    w1 = np.zeros((128, 129), np.float32)
    w1[:, 0:CL] = f("rel1W")
    w1[:, CL + 1:129] = f("att1W")

    whi = np.tile(f("att2W"), (1, CL + 1)).astype(np.float32)        # [64,65]
    aff0W = f("aff0W")
    wqe = np.zeros((CL + 1, CL), np.float32)
    wqe[0:CL] = (f("rel2W") + I) @ aff0W
    wqe[CL] = f("rel2b") @ aff0W                              # raff2

    sc1W = f("sc1W")
    aff2W, out0W, out1W = f("aff2W"), f("out0W"), f("out1W")
    wbi = np.zeros((CL, 8 * CL), np.float32)
    wbi[:, 0:CL] = f("sc0W")
    wbi[:, CL:2 * CL] = (sc1W + I) @ aff0W
    wbi[:, 3 * CL:4 * CL] = f("aff1W")
    wbi[:, 4 * CL:5 * CL] = aff2W @ out0W[0:CL]
    wbi[:, 5 * CL:6 * CL] = out0W[CL:128]
    wbi[:, 6 * CL:7 * CL] = out1W + I
    wbi[0, 7 * CL + 0] = 1.0
    wbi[1, 7 * CL + 1] = 1.0

    bias = np.zeros((128, 10), np.float32)
    bias[:, 0] = f("rel0b")
    bias[:, 1] = f("att0b")
    bias[0:CL, 2] = f("rel1b")
    bias[CL, 2] = 1.0
    bias[0:CL, 9] = f("att1b")
    bias[0:CL, 3] = f("sc0b")
    bias[0:CL, 4] = aff0W.T @ f("sc1b") + f("aff0b")
    bias[0:CL, 5] = f("aff1b")
    bias[0:CL, 6] = out0W[0:CL].T @ f("aff2b") + f("out0b")
    bias[0:CL, 7] = f("out1b")
    bias[0:CL + 1, 8] = float(f("att2b")[0])

    shared = dict(wrel0=wrel0, watt0=watt0, wdist=wdist, w1=w1, whi=whi,
                  wqe=wqe, wbi=wbi, bias=bias)

    in_maps = []
    for c in range(ncores):
        xs = x[c * b:(c + 1) * b, -1]
        xfm = xs.reshape(NT, 4).T.astype(np.float32)     # [4, 2048]
        s0 = encWp.T @ xfm + encbp[:, None]              # [64, 2048]
        in_maps.append(dict(shared, s0=np.ascontiguousarray(s0, np.float32)))
    return in_maps


def kernel(**inputs):
    num_rollout = int(inputs["num_rollout"])
    if num_rollout not in _CACHE:
        _CACHE[num_rollout] = build_program(num_rollout)
    nc = _CACHE[num_rollout]

    in_maps = _prep(inputs)
    res = run_bass_kernel_spmd(nc, in_maps, list(range(NCORES)))

    x = np.asarray(inputs["x"], np.float32)
    rollout = np.empty((NCORES * B, num_rollout, O, 4), np.float32)
    for c in range(NCORES):
        ro = res.results[c]["ro"]                        # [roll, 4, 2048]
        rollout[c * B:(c + 1) * B] = (
            ro.reshape(num_rollout, 4, B, O).transpose(2, 0, 3, 1))
    present = x.copy()
    return rollout, present
